# revision 11
# baseline (speedup 1.0000x reference)
"""Trainium2 Bass kernel for nn_DCP_LOSS (stain-deconvolution loss).

Data-parallel over batch: B=8 -> 8 NeuronCores, one batch item per core.
Each core computes, for its (input, target) image pair and both stains (h, d):
  - per-pixel fod via ln -> channel-mix (PE diag matmuls) -> exp x3 -> ln
  - binary masks (fod >= 0.3) written to DRAM
  - block sums of fod_relu (fod >= 0.15) via scalar_tensor_tensor accum
  - 20-bin weighted histogram via cumulative masked sums (STT accum)
Host combines the per-core scalar stats into the final loss (the "all-reduce"
of the scalar loss terms) and stacks the masks.

Mathematical restructuring (verified exact vs the reference formula):
  s' = sum_c HED_FROM_RGB[c, col] * ln(max(rgb_c, 1e-6))       (no /LOG_ADJ)
  grey-with-clips == G = sum_d exp(M[row, d] * s' + ln(coeff_d))  as far as
  fod = max(-log10(G + calib), 0) is concerned (all clips in the reference
  only matter when fod == 0 anyway, and fod==0 contributes 0 to every
  reduction).
"""

import math
import os
import sys

sys.path.insert(0, "/opt/trn_rl_repo")

import numpy as np

import concourse.bacc as bacc
import concourse.bass as bass
import concourse.mybir as mybir
import concourse.tile as tile
from concourse import bass_utils

f32 = mybir.dt.float32

# ---- constants (from the reference) ----
_RGB_FROM_HED = np.array(
    [[0.65, 0.7, 0.29], [0.07, 0.99, 0.11], [0.27, 0.57, 0.78]], dtype=np.float64
)
_HED_FROM_RGB = np.linalg.inv(_RGB_FROM_HED)
_COEFFS = np.array([0.2125, 0.7154, 0.0721], dtype=np.float64)
_CAL = 10.0 ** (-math.e)  # alpha = 1 for both stains
_T_FOD = 0.15
_T_MASK = 0.3
_NUM_BINS = 20
_BW32 = np.float32(math.e) / np.float32(20.0)
_INV_LN10 = 1.0 / math.log(10.0)

B, C, H, W = 8, 3, 1024, 1024
P = 128  # partitions

# stain-image order used for stats tensors: (image, stain)
# si 0: x/h   1: x/d   2: y/h   3: y/d
_SI = [(0, 0), (0, 1), (1, 0), (1, 1)]


def _diag_consts() -> np.ndarray:
    """7 x [128,128]: diag(w_c) for h (c=0..2), for d (c=0..2), identity."""
    out = np.zeros((7, P, P), dtype=np.float32)
    eye = np.eye(P, dtype=np.float32)
    for c in range(3):
        out[c] = np.float32(_HED_FROM_RGB[c, 0]) * eye
        out[3 + c] = np.float32(_HED_FROM_RGB[c, 2]) * eye
    out[6] = eye
    return out


def build_program(nc, Himg: int, Wimg: int):
    """Emit the per-core program. Returns names of inputs/outputs."""
    nbands = Himg // P
    mmchunk = min(512, Wimg)
    assert Himg % P == 0 and Wimg % mmchunk == 0

    x_d = nc.dram_tensor("x", (3, Himg, Wimg), f32, kind="ExternalInput")
    y_d = nc.dram_tensor("y", (3, Himg, Wimg), f32, kind="ExternalInput")
    diag_d = nc.dram_tensor("diags", (7, P, P), f32, kind="ExternalInput")

    masks_d = [
        nc.dram_tensor(n, (Himg, Wimg), f32, kind="ExternalOutput")
        for n in ("im_h", "im_d", "tm_h", "tm_d")
    ]
    # hist accum: per stain-image, per bin k, per band b -> column k*nbands+b
    hacc_d = nc.dram_tensor(
        "hacc", (4, P, _NUM_BINS * nbands), f32, kind="ExternalOutput"
    )
    # block accum: per stain-image, per band, per 4 col-blocks
    bacc_d = nc.dram_tensor("bacc", (4, P, nbands * 4), f32, kind="ExternalOutput")

    cbw = Wimg // 4  # col-block width (256 for full size)
    m_h = [np.float32(_RGB_FROM_HED[0, d]) for d in range(3)]
    m_d = [np.float32(_RGB_FROM_HED[2, d]) for d in range(3)]
    lnc = [float(np.float32(math.log(_COEFFS[d]))) for d in range(3)]
    edges = [float(np.float32(k) * _BW32) for k in range(_NUM_BINS)]

    with tile.TileContext(nc) as tc:
        with (
            tc.tile_pool(name="const", bufs=1) as constp,
            tc.tile_pool(name="chan", bufs=2) as chanp,
            tc.tile_pool(name="epool", bufs=2) as epool,
            tc.tile_pool(name="upool", bufs=2) as upool,
            tc.tile_pool(name="fodp", bufs=2) as fodp,
            tc.tile_pool(name="maskp", bufs=2) as maskp,
            tc.tile_pool(name="accp", bufs=1) as accp,
            tc.tile_pool(name="scr", bufs=1) as scrp,
            tc.tile_pool(name="psum", bufs=2, space="PSUM") as psump,
        ):
            diags = []
            for i in range(7):
                dt_ = constp.tile([P, P], f32, tag=f"diag{i}", name=f"diag{i}")
                nc.sync.dma_start(dt_[:], diag_d[i])
                diags.append(dt_)

            hacc_sb = [accp.tile([P, _NUM_BINS * nbands], f32, tag=f"hacc{i}", name=f"hacc{i}")
                       for i in range(4)]
            bacc_sb = [accp.tile([P, nbands * 4], f32, tag=f"bacc{i}", name=f"bacc{i}")
                       for i in range(4)]
            scratch = scrp.tile([P, Wimg], f32)

            bias_lnc = []
            for d in range(3):
                bt = constp.tile([P, 1], f32, tag=f"blnc{d}", name=f"blnc{d}")
                nc.vector.memset(bt[:], lnc[d])
                bias_lnc.append(bt)
            bias_cal = constp.tile([P, 1], f32)
            nc.vector.memset(bias_cal[:], float(_CAL))

            for img_i, img_d in ((0, x_d), (1, y_d)):
                for b in range(nbands):
                    rows = slice(b * P, (b + 1) * P)
                    # load 3 channel tiles, clamp, ln (in-place)
                    L = []
                    for c in range(3):
                        t = chanp.tile([P, Wimg], f32, tag=f"ch{c}", name=f"ch{c}")
                        nc.sync.dma_start(t[:], img_d[c, rows, :])
                        nc.vector.tensor_scalar_max(t[:], t[:], 1e-6)
                        nc.scalar.activation(
                            t[:], t[:], mybir.ActivationFunctionType.Ln
                        )
                        L.append(t)

                    for st in range(2):  # 0=h, 1=d
                        si = img_i * 2 + st
                        mvals = m_h if st == 0 else m_d
                        # s' = sum_c diag(w_c) @ L_c   (PSUM accumulate)
                        sp = psump.tile([P, Wimg], f32, tag="sp")
                        for c2 in range(Wimg // mmchunk):
                            cols = slice(c2 * mmchunk, (c2 + 1) * mmchunk)
                            for c in range(3):
                                nc.tensor.matmul(
                                    sp[:, cols],
                                    diags[3 * st + c][:],
                                    L[c][:, cols],
                                    start=(c == 0),
                                    stop=(c == 2),
                                )
                        # e_d = exp(m_d * s' + ln c_d)
                        E = []
                        for d in range(3):
                            e = epool.tile([P, Wimg], f32, tag=f"e{d}", name=f"e{d}")
                            nc.scalar.activation(
                                e[:],
                                sp[:],
                                mybir.ActivationFunctionType.Exp,
                                bias=bias_lnc[d][:],
                                scale=float(mvals[d]),
                            )
                            E.append(e)
                        # G = e0 + e1 + e2  (PSUM accumulate via identity)
                        G = psump.tile([P, Wimg], f32, tag="G")
                        for c2 in range(Wimg // mmchunk):
                            cols = slice(c2 * mmchunk, (c2 + 1) * mmchunk)
                            for d in range(3):
                                nc.tensor.matmul(
                                    G[:, cols],
                                    diags[6][:],
                                    E[d][:, cols],
                                    start=(d == 0),
                                    stop=(d == 2),
                                )
                        # u = ln(G + calib)
                        u = upool.tile([P, Wimg], f32)
                        nc.scalar.activation(
                            u[:], G[:], mybir.ActivationFunctionType.Ln,
                            bias=bias_cal[:],
                        )
                        # fod = max(-u/ln10, 0)
                        fod = fodp.tile([P, Wimg], f32)
                        nc.vector.tensor_scalar(
                            fod[:], u[:], -float(_INV_LN10), 0.0,
                            mybir.AluOpType.mult, mybir.AluOpType.max,
                        )
                        # mask = fod >= 0.3
                        mk = maskp.tile([P, Wimg], f32)
                        nc.vector.tensor_scalar(
                            mk[:], fod[:], float(np.float32(_T_MASK)), None,
                            mybir.AluOpType.is_ge,
                        )
                        nc.sync.dma_start(masks_d[si][rows, :], mk[:])
                        # block sums: (fod >= 0.15) * fod per col-block
                        for cb in range(4):
                            cols = slice(cb * cbw, (cb + 1) * cbw)
                            nc.vector.scalar_tensor_tensor(
                                scratch[:, cols],
                                fod[:, cols],
                                float(np.float32(_T_FOD)),
                                fod[:, cols],
                                mybir.AluOpType.is_ge,
                                mybir.AluOpType.mult,
                                accum_out=bacc_sb[si][:, b * 4 + cb : b * 4 + cb + 1],
                            )
                        # histogram cumulative masked sums M_k
                        for k in range(_NUM_BINS):
                            nc.vector.scalar_tensor_tensor(
                                scratch[:],
                                fod[:],
                                edges[k],
                                fod[:],
                                mybir.AluOpType.is_ge,
                                mybir.AluOpType.mult,
                                accum_out=hacc_sb[si][
                                    :, k * nbands + b : k * nbands + b + 1
                                ],
                            )

            for si in range(4):
                nc.sync.dma_start(hacc_d[si], hacc_sb[si][:])
                nc.sync.dma_start(bacc_d[si], bacc_sb[si][:])

    return dict(
        inputs=("x", "y", "diags"),
        outputs=("im_h", "im_d", "tm_h", "tm_d", "hacc", "bacc"),
        nbands=nbands,
    )


# --------------------------------------------------------------------------
# host-side finishing
# --------------------------------------------------------------------------


def _finish_stats(hacc: np.ndarray, bacc: np.ndarray, nbands: int):
    """hacc [4,128,20*nbands], bacc [4,128,4*nbands] ->
    per stain-image: hist [20], blocks [4,4], avg (f64).

    bacc partition p of band b holds the (fod>=0.15)*fod sum of image row
    b*128+p for each of the 4 col-blocks; rows map to the 4 row-blocks by
    global_row // (H/4)."""
    Himg = nbands * P
    rows_per_block = Himg // 4
    grow = (np.arange(nbands * P) // rows_per_block)  # [nbands*P] -> block row
    hists, blocks, avgs = [], [], []
    for si in range(4):
        M = hacc[si].reshape(P, _NUM_BINS, nbands).sum(axis=(0, 2), dtype=np.float64)
        bins = M.copy()
        bins[:-1] -= M[1:]
        hists.append(bins)
        # bacc[si]: [P, nbands, 4] -> per-global-row [nbands*P, 4]
        per_row = bacc[si].reshape(P, nbands, 4).transpose(1, 0, 2).reshape(-1, 4)
        blk = np.zeros((4, 4), dtype=np.float64)
        for r in range(4):
            blk[r] = per_row[grow == r].sum(axis=0, dtype=np.float64)
        blocks.append(blk)
        avgs.append(blk.sum())
    return hists, blocks, avgs


def _channel_loss(i_avg, i_blk, i_his, t_avg, t_blk, t_his, Bsz, HWsz):
    """numpy (f64) mirror of the reference _channel_loss.
    i_avg/t_avg: [B]; i_blk/t_blk: [B,4,4]; i_his/t_his: [B,20]."""
    avg_t = (i_avg - t_avg) ** 2 / float(HWsz) ** 2
    his_t = np.sum((i_his / HWsz - t_his / HWsz) ** 2, axis=1) / Bsz
    blk_t = np.mean((i_blk / (HWsz / 16.0) - t_blk / (HWsz / 16.0)) ** 2)
    diff = i_avg - t_avg
    cond = (diff >= t_avg * -0.4) & (diff <= t_avg * 0.4)
    return np.sum(np.where(cond, his_t, avg_t + his_t)) + blk_t


_BUILT = {}
LAST_RESULTS = None


def _get_compiled():
    key = (H, W)
    if key not in _BUILT:
        nc = bacc.Bacc("TRN2", target_bir_lowering=False, debug=False)
        build_program(nc, H, W)
        nc.compile()
        _BUILT[key] = nc
    return _BUILT[key]


def kernel(inputs: np.ndarray, targets: np.ndarray):
    inputs = np.ascontiguousarray(np.asarray(inputs, dtype=np.float32))
    targets = np.ascontiguousarray(np.asarray(targets, dtype=np.float32))
    assert inputs.shape == (B, C, H, W)

    nc = _get_compiled()
    diags = _diag_consts()
    in_maps = [
        {"x": inputs[b], "y": targets[b], "diags": diags} for b in range(B)
    ]
    trace = bool(int(os.environ.get("TRN_KERNEL_TRACE", "0")))
    res = bass_utils.run_bass_kernel_spmd(
        nc, in_maps, core_ids=list(range(B)), trace=trace
    )
    global LAST_RESULTS
    LAST_RESULTS = res
    results = res.results

    nbands = H // P
    im_h = np.stack([results[b]["im_h"] for b in range(B)])
    im_d = np.stack([results[b]["im_d"] for b in range(B)])
    tm_h = np.stack([results[b]["tm_h"] for b in range(B)])
    tm_d = np.stack([results[b]["tm_d"] for b in range(B)])

    # gather per-batch stats: [B] x (hists, blocks, avgs) for si in 0..3
    ia = np.zeros((4, B)); ih = np.zeros((4, B, _NUM_BINS)); ib = np.zeros((4, B, 4, 4))
    for b in range(B):
        hists, blocks, avgs = _finish_stats(
            results[b]["hacc"], results[b]["bacc"], nbands
        )
        for si in range(4):
            ia[si, b] = avgs[si]
            ih[si, b] = hists[si]
            ib[si, b] = blocks[si]

    HWsz = H * W
    # si order: 0=x_h 1=x_d 2=y_h 3=y_d
    loss = _channel_loss(ia[0], ib[0], ih[0], ia[2], ib[2], ih[2], B, HWsz) + \
        _channel_loss(ia[1], ib[1], ih[1], ia[3], ib[3], ih[3], B, HWsz)

    return (np.float32(loss), im_h, tm_h, im_d, tm_d)
